# revision 9
# baseline (speedup 1.0000x reference)
"""Distributed multi-head attention kernel for Trainium2 (8 NeuronCores).

Problem: B=2, S=2048, D=1024, H=16 heads, DH=64.
  qkv = x @ w_qkv + b_qkv ; per-head softmax(q k^T / 8) v ; out proj.

Sharding: core c = g*4 + j handles batch g and heads 4j..4j+3.

Stage-B schedule:
  - Head-outer attention: for (pair, head, q-half) process 16 k-tiles;
    each quarter's [64, 1024] attention output is normalized and
    AllToAll'd immediately as a [8, 64, 128] exchange (128 KiB), so 7
    of the 8 collectives fully overlap attention and only the last
    ~8us is exposed (vs. one 512 KiB collective at the tail).
  - Dest mapping: core c owns q rows {qh*1024 + c*128} of each batch,
    so every quarter's exchange carries data for all 8 peers.
  - Fillers keep the PE ahead of the ACT exp stream: v tiles 2-15
    paced into quarter 0, the pair-1 q/k projection into quarters 1-2.
  - aout bounce DMAs are batched two quarters behind their collective
    so the sync queue never blocks on an in-flight collective.
  - Output staged and DMA'd as bf16 (host upcasts).

Layout trick: scores are computed transposed (scoresT[k,q] = kT.T@qT),
the exp output feeds attn@v as the *moving* operand, and a ones column
appended to v yields the softmax row-sums as a 65th output row of the
same matmul.  Matmul operands bf16, fp32 PSUM accumulation.
"""

import numpy as np

import concourse.bacc as bacc
import concourse.mybir as mybir
import concourse.tile as tile
from concourse import bass_utils

F32 = mybir.dt.float32
BF16 = mybir.dt.bfloat16
EXP = mybir.ActivationFunctionType.Exp
MULT = mybir.AluOpType.mult

B, S, D, H = 2, 2048, 1024, 16
DH = D // H            # 64
NCORE = 8
GRP = 4                # cores per batch group
HL = H // GRP          # 4 local heads per core
DTILES = D // 128      # 8 contraction chunks
STILES = S // 128      # 16
QB = 128               # per-dest q block width
VW = DH + 1            # 65: v columns + ones column
VP = 128               # padded v block: [v(64) | ones(1) | zeros(63)]
NQ = 8                 # attention quarters (pr, h, qh)

_CACHE = {}


def _build():
    nc = bacc.Bacc("TRN2", target_bir_lowering=False, debug=False,
                   num_devices=NCORE)

    xT_d = nc.dram_tensor("xT", [D, S], BF16, kind="ExternalInput")
    wqk_d = nc.dram_tensor("wqk", [D, 2 * HL * DH], BF16, kind="ExternalInput")
    wv_d = nc.dram_tensor("wv", [D, HL * DH], BF16, kind="ExternalInput")
    bqk_d = nc.dram_tensor("bqk", [2 * HL * DH], F32, kind="ExternalInput")
    bv_d = nc.dram_tensor("bv", [HL * DH], F32, kind="ExternalInput")
    wout_d = nc.dram_tensor("wout", [D, D], BF16, kind="ExternalInput")
    bout_d = nc.dram_tensor("bout", [D], F32, kind="ExternalInput")
    ident_d = nc.dram_tensor("ident", [128, 128], BF16, kind="ExternalInput")
    out_d = nc.dram_tensor("out", [4 * QB, D], BF16, kind="ExternalOutput")

    groups = [list(range(NCORE))]

    with tile.TileContext(nc) as tc:
        with (
            tc.tile_pool(name="persist", bufs=1) as pers,
            tc.tile_pool(name="big", bufs=DTILES) as big,
            tc.tile_pool(name="wsmall", bufs=1) as wsmall,
            tc.tile_pool(name="ppool", bufs=8) as ppool,
            tc.tile_pool(name="npool", bufs=2) as npool,
            tc.tile_pool(name="fin", bufs=2) as fin,
            tc.tile_pool(name="dram", bufs=1, space="DRAM") as dram,
        ):
            # ---- persistent SBUF tensors ----
            kT = pers.tile([128, 2 * S], BF16, tag="kT")
            qp = pers.tile([128, 4 * S], BF16, tag="qp")
            vext = pers.tile([128, STILES * HL * VP], BF16, tag="vext")
            # aout: 32 col-blocks of 128: block b = pr*16 + 2*jj + qh,
            # rows h*64..h*64+64 filled by quarter (pr, h, qh)'s exchange.
            aout = pers.tile([128, 32 * QB], BF16, tag="aout")
            outacc = pers.tile([128, 4 * D], BF16, tag="outacc")
            bqk_sb = pers.tile([128, 4], F32, tag="bqk_sb")
            bv_sb = pers.tile([128, HL * DH], F32, tag="bv_sb")
            bout_bf = pers.tile([128, D], BF16, tag="bout_bf")
            bout_row = ppool.tile([1, D], F32, tag="P", name="bout_row")
            ident = pers.tile([128, 128], BF16, tag="ident")
            e0m = pers.tile([128, 128], BF16, tag="e0m")

            wqk_sb = wsmall.tile([128, DTILES * 512], BF16, tag="wqk_sb")
            wv_sb = wsmall.tile([128, DTILES * 256], BF16, tag="wv_sb")
            wout_sb = wsmall.tile([128, DTILES * D], BF16, tag="wout_sb")

            # ---- heater seed + all pure memsets first on DVE ----
            heat_b = wsmall.tile([128, 512], BF16, tag="heat_b")
            nc.vector.memset(heat_b[:], 0.25)
            nc.vector.memset(bout_bf[:], 0.0)
            nc.vector.memset(e0m[:], 0.0)
            nc.vector.memset(e0m[0:1, :], 1.0)
            nc.vector.memset(
                vext[:].rearrange("p (b w) -> p b w", w=VP)[:, :, DH + 1:VP],
                0.0)
            nc.vector.memset(
                vext[:].rearrange("p (b w) -> p b w", w=VP)[:, :, DH:DH + 1],
                1.0)
            for pr in range(2):
                nc.vector.memset(qp[64:128, (2 * pr) * S:(2 * pr + 1) * S], 0.0)
                nc.vector.memset(qp[0:64, (2 * pr + 1) * S:(2 * pr + 2) * S], 0.0)

            # ---- input DMAs.  The phase-A critical wqk/x stream issues
            # FIRST on both queues (each descriptor costs ~0.7us of issue
            # time on its engine); small transfers follow on sync (needed
            # only once phase A drains); wout (tail-only) last. ----
            engs = [nc.sync, nc.gpsimd]
            xt_tiles = [big.tile([128, S], BF16, tag="big", name=f"xt{dt}")
                        for dt in range(DTILES)]
            for dt in range(DTILES):
                q = engs[0] if dt < 4 else engs[1]
                q.dma_start(
                    wqk_sb[:, dt * 512:(dt + 1) * 512],
                    wqk_d[dt * 128:(dt + 1) * 128, :])
                q.dma_start(xt_tiles[dt][:], xT_d[dt * 128:(dt + 1) * 128, :])
            DT_ORDER = [0, 4, 1, 5, 2, 6, 3, 7]
            engs[1].dma_start(
                wv_sb[:].rearrange("p (d c) -> p d c", c=256),
                wv_d[:].rearrange("(d p) c -> p d c", p=128))
            # preload the gpsimd custom-op library (partition_broadcast's
            # ~15us LOAD_LIB) while the PE runs phase A; scratch target.
            lib_warm = wsmall.tile([64, 64], F32, tag="lib_warm")
            heat_f32 = wsmall.tile([1, 64], F32, tag="heat_f32")
            nc.vector.memset(heat_f32[:], 1.0)
            nc.gpsimd.partition_broadcast(lib_warm[:], heat_f32[:1, :])
            # small transfers: bqk/bv needed at phase-A drain (~25us),
            # bout/ident only at the tail.  bv_sb broadcast is done by the
            # DMA itself (0-stride source), not a gpsimd custom op.
            nc.sync.dma_start(bqk_sb[:],
                              bqk_d[:].rearrange("(e p) -> p e", p=128))
            nc.sync.dma_start(
                bv_sb[:], bv_d[:].unsqueeze(0).broadcast_to([128, HL * DH]))
            nc.sync.dma_start(bout_row[:], bout_d[:].unsqueeze(0))
            nc.sync.dma_start(ident[:], ident_d[:])
            engs[0].dma_start(
                wout_sb[:].rearrange("p (d c) -> p d c", c=D),
                wout_d[:].rearrange("(d p) c -> p d c", p=128))
            nc.vector.tensor_copy(bout_bf[0:1, :], bout_row[:1, :])

            # ---- pre-warm heater ----
            NHEAT = 40
            if NHEAT:
                with tc.tile_pool(name="psH", bufs=1, space="PSUM") as psH:
                    ph = psH.tile([128, 512], F32, tag="psH")
                    for i in range(NHEAT):
                        nc.tensor.matmul(ph[:, 0:256], heat_b[:, 0:128],
                                         heat_b[:, 0:256],
                                         start=True, stop=True)

            # ---- projection helpers ----
            def qk_store(et, sh, acc):
                dsts = slice(sh * 1024, (sh + 1) * 1024)
                if et >= 2:
                    pr = et - 2
                    nc.vector.tensor_scalar_add(
                        kT[:, pr * S:(pr + 1) * S][:, dsts],
                        acc[:], bqk_sb[:, et:et + 1])
                else:
                    pr = et
                    nc.vector.tensor_scalar_add(
                        qp[0:64, (2 * pr) * S:(2 * pr + 1) * S][:, dsts],
                        acc[0:64, :], bqk_sb[0:64, et:et + 1])
                    nc.vector.tensor_scalar_add(
                        qp[64:128, (2 * pr + 1) * S:(2 * pr + 2) * S][:, dsts],
                        acc[64:128, :], bqk_sb[64:128, et:et + 1])

            def heat(pool, tag, n, cols=512):
                # p-state keep-warm: dependency-free matmuls into a scratch
                # PSUM tile (fresh alloc so pool rotation stays shallow)
                hacc = pool.tile([128, 512], F32, tag=tag, name="hgap")
                for _ in range(n):
                    nc.tensor.matmul(hacc[:, 0:cols], heat_b[:, 0:128],
                                     heat_b[:, 0:cols], start=True, stop=True)

            def qk_chunk(pool, tag, et, sh, c):
                acc = pool.tile([128, 512], F32, tag=tag,
                                name=f"qk{et}{sh}{c}")
                sl = slice(sh * 1024 + c * 512, sh * 1024 + (c + 1) * 512)
                for dt in range(DTILES):
                    nc.tensor.matmul(
                        acc[:],
                        wqk_sb[:, dt * 512 + et * 128:
                               dt * 512 + (et + 1) * 128],
                        xt_tiles[dt][:, sl],
                        start=(dt == 0), stop=(dt == DTILES - 1))
                if et >= 2:
                    pr = et - 2
                    nc.vector.tensor_scalar_add(
                        kT[:, pr * S:(pr + 1) * S][:, sl],
                        acc[:], bqk_sb[:, et:et + 1])
                else:
                    pr = et
                    nc.vector.tensor_scalar_add(
                        qp[0:64, (2 * pr) * S:(2 * pr + 1) * S][:, sl],
                        acc[0:64, :], bqk_sb[0:64, et:et + 1])
                    nc.vector.tensor_scalar_add(
                        qp[64:128, (2 * pr + 1) * S:(2 * pr + 2) * S][:, sl],
                        acc[64:128, :], bqk_sb[64:128, et:et + 1])

            def v_acc(pool, tag, st):
                acc = pool.tile([128, HL * DH], F32, tag=tag, name=f"v{st}")
                for dt in range(DTILES):
                    nc.tensor.matmul(
                        acc[:],
                        xt_tiles[dt][:, st * 128:(st + 1) * 128],
                        wv_sb[:, dt * 256:(dt + 1) * 256],
                        start=(dt == 0), stop=(dt == DTILES - 1))
                base = st * HL * VP
                vv = vext[:, base:base + HL * VP].rearrange(
                    "p (h w) -> p h w", h=HL)
                nc.vector.tensor_add(
                    vv[:, :, 0:DH],
                    acc[:].rearrange("p (h w) -> p h w", h=HL),
                    bv_sb[:].rearrange("p (h w) -> p h w", h=HL))

            # ---- phase A: k pair 0 (both halves) + q pair 0 first half,
            # 3 live accumulators consumed per-dt as DMAs land; heat
            # trickles between dt groups keep the p-state up through the
            # DMA-paced ramp.  (et0, sh1) is paced into quarter 0 as two
            # qk_chunk fillers.  Drains ordered so quarter 0's needs
            # (k-sh0, q-sh0) clear the DVE first. ----
            PHA = [(2, 0), (0, 0), (2, 1)]
            with (
                tc.tile_pool(name="psA0", bufs=3, space="PSUM") as psA0,
                tc.tile_pool(name="psH2", bufs=1, space="PSUM") as psH2,
            ):
                accs = {k: psA0.tile([128, 1024], F32, tag="psA0",
                                     name=f"qa{k[0]}{k[1]}")
                        for k in PHA}
                hacc = psH2.tile([128, 512], F32, tag="psH2", name="hacc")
                for i, dt in enumerate(DT_ORDER):
                    last = i == DTILES - 1
                    for et, sh in PHA:
                        for c in range(2):
                            sl = slice(sh * 1024 + c * 512,
                                       sh * 1024 + (c + 1) * 512)
                            nc.tensor.matmul(
                                accs[(et, sh)][:, c * 512:(c + 1) * 512],
                                wqk_sb[:, dt * 512 + et * 128:
                                       dt * 512 + (et + 1) * 128],
                                xt_tiles[dt][:, sl],
                                start=(i == 0), stop=last)
                        if last:
                            qk_store(et, sh, accs[(et, sh)])
                    if not last:
                        for _ in range(5):
                            nc.tensor.matmul(hacc[:], heat_b[:, 0:128],
                                             heat_b[:], start=True, stop=True)

            wout_tiles = [wout_sb[:, ec * D:(ec + 1) * D]
                          for ec in range(DTILES)]

            # ---- attention: 8 quarters (pr, h, qh), one AllToAll each ----
            a2a_in = [dram.tile([NCORE, 64, QB], BF16, tag=f"a2a_in{q}",
                                name=f"a2a_in{q}") for q in range(NQ)]
            a2a_out = [dram.tile([NCORE, 64, QB], BF16, tag=f"a2a_out{q}",
                                 name=f"a2a_out{q}") for q in range(NQ)]

            def aout_land(qi):
                # pull exchange qi's result into aout (rows h*64..,
                # col blocks pr*16 + 2*jj + qh for jj in 0..7)
                pr, h, qh = qi >> 2, (qi >> 1) & 1, qi & 1
                dst = aout[h * 64:(h + 1) * 64, :].rearrange(
                    "p (a j q c) -> p a j q c", a=2, j=NCORE, q=2)[:, pr, :, qh, :]
                nc.sync.dma_start(
                    dst, a2a_out[qi][:].rearrange("d p c -> p d c"))

            # psS: 3 slots so filler allocations don't collapse the
            # scores->exp double-buffering; po single-buffered (outproj
            # accumulators borrow psS slots at the tail).
            with (
                tc.tile_pool(name="psS", bufs=3, space="PSUM") as psS,
                tc.tile_pool(name="psO", bufs=1, space="PSUM") as psO,
            ):
                # v tiles 0,1 ahead of the loop
                for st in range(2):
                    v_acc(psS, "psS", st)

                def outproj_block(p, gb, qb):
                    acc = psS.tile([128, D], F32, tag="psS",
                                   name=f"op{gb}{qb}")
                    oa = outacc[:, (gb * 2 + qb) * D:(gb * 2 + qb + 1) * D]
                    for c in range(2):
                        nc.tensor.matmul(
                            acc[:, c * 512:(c + 1) * 512],
                            e0m[:] if p == 0 else ident[:],
                            (bout_bf if p == 0 else oa)
                            [:, c * 512:(c + 1) * 512],
                            start=True, stop=False)
                    for jr in range(GRP):
                        jj = gb * GRP + jr
                        blk = p * 16 + 2 * jj + qb
                        for c in range(2):
                            nc.tensor.matmul(
                                acc[:, c * 512:(c + 1) * 512],
                                aout[:, blk * QB:(blk + 1) * QB],
                                wout_tiles[p * GRP + jr][:, c * 512:(c + 1) * 512],
                                start=False, stop=(jr == GRP - 1))
                    if p == 0:
                        nc.scalar.copy(oa, acc[:])
                    else:
                        res = fin.tile([128, D], BF16, tag="res")
                        nc.scalar.copy(res[:], acc[:])
                        row = (gb * 2 + qb) * QB
                        (nc.sync if gb == 0 else nc.gpsimd).dma_start(
                            out_d[row:row + QB, :], res[:])

                # filler schedule: quarter 0 -> v tiles 2..15 at kt 0..13
                # and the deferred (et0, sh1) q-projection at kt 14..15;
                # quarters 1,2 -> pair-1 qk chunks at kt 1,5,9,13;
                # quarter 7 -> outproj pair-0 bursts at kt 2,6,10,14.
                # Every other iteration gets one 512-col heat matmul so the
                # PE never idles waiting on the exp stream (idle drops the
                # p-state and with it the matmul clock).
                qkc = [(et, sh, c) for et in (3, 1) for sh in range(2)
                       for c in range(2)]

                for qi in range(NQ):
                    pr, h, qh = qi >> 2, (qi >> 1) & 1, qi & 1
                    lh = 2 * pr + h
                    po = psO.tile([128, 1024], F32, tag="psO", name="po")
                    prev_p = None
                    for kt in range(STILES):
                        ps = psS.tile([128, 1024], F32, tag="psS", name="ps")
                        filler = (
                            qi == 0 or (qi in (1, 2) and kt % 4 == 1) or
                            (qi == NQ - 1 and kt % 4 == 2))
                        if not filler:
                            # p-state keep-warm into the fresh ps tile; the
                            # scores matmul below overwrites (start=True).
                            # Shares the slot wait, so no extra stalls.
                            nc.tensor.matmul(ps[:, 0:512], heat_b[:, 0:128],
                                             heat_b[:], start=True, stop=True)
                        for c in range(2):
                            nc.tensor.matmul(
                                ps[:, c * 512:(c + 1) * 512],
                                kT[:, pr * S + kt * 128:
                                   pr * S + (kt + 1) * 128],
                                qp[:, lh * S + qh * 1024 + c * 512:
                                   lh * S + qh * 1024 + (c + 1) * 512],
                                start=True, stop=True)
                        if prev_p is not None:
                            vb = ((kt - 1) * HL + lh) * VP
                            for c in range(2):
                                nc.tensor.matmul(
                                    po[:, c * 512:(c + 1) * 512],
                                    vext[:, vb:vb + VP],
                                    prev_p[:, c * 512:(c + 1) * 512],
                                    start=(kt - 1 == 0), stop=False)
                        pexp = ppool.tile([128, 1024], BF16, tag="P",
                                          name="pexp")
                        nc.scalar.activation(pexp[:], ps[:], EXP, scale=0.125)
                        prev_p = pexp
                        # fillers
                        if qi == 0 and kt < 14:
                            v_acc(psS, "psS", kt + 2)
                        elif qi == 0:
                            qk_chunk(psS, "psS", 0, 1, kt - 14)
                        elif qi in (1, 2) and kt % 4 == 1:
                            et, sh, c = qkc[(qi - 1) * 4 + kt // 4]
                            qk_chunk(psS, "psS", et, sh, c)
                        elif qi == NQ - 1 and kt % 4 == 2:
                            i = kt // 4
                            outproj_block(0, i // 2, i % 2)
                    vb = ((STILES - 1) * HL + lh) * VP
                    for c in range(2):
                        nc.tensor.matmul(
                            po[:, c * 512:(c + 1) * 512],
                            vext[:, vb:vb + VP],
                            prev_p[:, c * 512:(c + 1) * 512],
                            start=False, stop=True)

                    last_q = qi == NQ - 1

                    # normalize quarter -> attn [64, 1024] bf16
                    rs_row = npool.tile([1, 1024], F32, tag="rs_row",
                                        name="rs_row")
                    rs_rec = npool.tile([1, 1024], F32, tag="rs_rec",
                                        name="rs_rec")
                    rs_b = npool.tile([64, 1024], F32, tag="rs_b",
                                      name="rs_b")
                    attn = npool.tile([64, 1024], BF16, tag="attn",
                                      name="attn")
                    nc.vector.tensor_copy(rs_row[:], po[DH:VW, :])
                    nc.vector.reciprocal_approx_fast(rs_rec[:], rs_row[:1, :])
                    nc.gpsimd.partition_broadcast(rs_b[:], rs_rec[:1, :])
                    if last_q:
                        nc.vector.tensor_tensor(attn[:], po[0:DH, :],
                                                rs_b[:], MULT)
                    else:
                        stg = npool.tile([DH, 1024], F32, tag="stg",
                                         name="stg")
                        nc.vector.tensor_copy(stg[:], po[0:DH, :])
                        nc.vector.tensor_tensor(attn[:], stg[:, :],
                                                rs_b[:], MULT)
                    # exchange input: one descriptor, 8 dest slices of 128
                    nc.sync.dma_start(
                        a2a_in[qi][:].rearrange("d p c -> p d c"),
                        attn[:].rearrange("p (d c) -> p d c", c=QB))
                    nc.gpsimd.collective_compute(
                        "AllToAll", mybir.AluOpType.bypass,
                        replica_groups=groups,
                        ins=[a2a_in[qi][:].opt()],
                        outs=[a2a_out[qi][:].opt()])
                    # land results late enough that the sync queue never
                    # blocks on an in-flight collective (the CC stream can
                    # back up ~40-100us behind the init barrier): pair-0
                    # at quarters 5/6 (needed by the quarter-7 outproj
                    # bursts), pair-1 at the tail.
                    if qi == 5:
                        aout_land(0), aout_land(1)
                    elif qi == 6:
                        aout_land(2), aout_land(3), aout_land(4)
                    elif qi == 7:
                        aout_land(5)
                        heat(psS, "psS", 55)
                        aout_land(6)
                        aout_land(7)

                for gb in range(2):
                    for qb in range(2):
                        outproj_block(1, gb, qb)

    nc.compile()
    return nc


def _shard(inputs):
    import ml_dtypes
    bf = ml_dtypes.bfloat16
    x = np.asarray(inputs["x"], np.float32)
    w_qkv = np.asarray(inputs["w_qkv"], np.float32)
    b_qkv = np.asarray(inputs["b_qkv"], np.float32)
    w_out = np.asarray(inputs["w_out"], np.float32)
    b_out = np.asarray(inputs["b_out"], np.float32)

    # wout rows permuted to match aout row order: for pair p, peer
    # rank-in-group jr, t in (0,1): head 4*jr + 2*p + t
    rows = []
    for p in (0, 1):
        for jr in range(GRP):
            for t in (0, 1):
                h = 4 * jr + 2 * p + t
                rows.append(w_out[h * DH:(h + 1) * DH, :])
    wout_perm = np.ascontiguousarray(np.concatenate(rows, 0))

    in_maps = []
    for c in range(NCORE):
        g, j = c // GRP, c % GRP
        cs = slice(j * HL * DH, (j + 1) * HL * DH)
        wqk = np.concatenate([w_qkv[:, :D][:, cs], w_qkv[:, D:2 * D][:, cs]], 1)
        bqk = np.concatenate([b_qkv[:D][cs], b_qkv[D:2 * D][cs]])
        in_maps.append({
            "xT": np.ascontiguousarray(x[g].T).astype(bf),
            "wqk": np.ascontiguousarray(wqk).astype(bf),
            "wv": np.ascontiguousarray(w_qkv[:, 2 * D:][:, cs]).astype(bf),
            "bqk": np.ascontiguousarray(bqk),
            "bv": np.ascontiguousarray(b_qkv[2 * D:][cs]),
            "wout": wout_perm.astype(bf),
            "bout": b_out,
            "ident": np.eye(128, dtype=np.float32).astype(bf),
        })
    return in_maps


def _install_ntff_hook():
    """The agent image's antenv lacks axon_hooks; shim it and register the
    ctypes NTFF profiler from trn_agent_boot so trace=True works."""
    import sys
    import types

    if "antenv.axon_hooks" in sys.modules:
        return
    import antenv

    mod = types.ModuleType("antenv.axon_hooks")
    mod._hook = None
    mod.set_axon_ntff_profile_hook = lambda h: setattr(mod, "_hook", h)
    mod.get_axon_ntff_profile_hook = lambda: mod._hook
    sys.modules["antenv.axon_hooks"] = mod
    antenv.axon_hooks = mod
    try:
        from trn_agent_boot.trn_boot import _ntff_profile_via_ctypes
        mod._hook = _ntff_profile_via_ctypes("/opt/axon/libaxon_pjrt.so")
    except Exception as e:
        print(f"ntff hook install failed: {e}")


def _run(inputs, trace=False):
    if trace:
        _install_ntff_hook()
    if "nc" not in _CACHE:
        _CACHE["nc"] = _build()
    nc = _CACHE["nc"]
    in_maps = _shard(inputs)
    r = bass_utils.run_bass_kernel_spmd(
        nc, in_maps, core_ids=list(range(NCORE)), trace=trace)
    out = np.empty((B, S, D), np.float32)
    for c in range(NCORE):
        for g in range(B):
            for qh in range(2):
                out[g, qh * 1024 + c * QB: qh * 1024 + (c + 1) * QB, :] = \
                    r.results[c]["out"][(g * 2 + qh) * QB:
                                        (g * 2 + qh + 1) * QB].astype(np.float32)
    return out, r


def kernel(**inputs) -> np.ndarray:
    out, _ = _run(inputs, trace=False)
    return out


# revision 10
# speedup vs baseline: 1.0784x; 1.0784x over previous
"""Distributed multi-head attention kernel for Trainium2 (8 NeuronCores).

Problem: B=2, S=2048, D=1024, H=16 heads, DH=64.
  qkv = x @ w_qkv + b_qkv ; per-head softmax(q k^T / 8) v ; out proj.

Sharding: core c = g*4 + j handles batch g and heads 4j..4j+3.

Stage-B schedule:
  - Head-outer attention: for (pair, head, q-half) process 16 k-tiles;
    each quarter's [64, 1024] attention output is normalized and
    AllToAll'd immediately as a [8, 64, 128] exchange (128 KiB), so 7
    of the 8 collectives fully overlap attention and only the last
    ~8us is exposed (vs. one 512 KiB collective at the tail).
  - Dest mapping: core c owns q rows {qh*1024 + c*128} of each batch,
    so every quarter's exchange carries data for all 8 peers.
  - Fillers keep the PE ahead of the ACT exp stream: v tiles 2-15
    paced into quarter 0, the pair-1 q/k projection into quarters 1-2.
  - aout bounce DMAs are batched two quarters behind their collective
    so the sync queue never blocks on an in-flight collective.
  - Output staged and DMA'd as bf16 (host upcasts).

Layout trick: scores are computed transposed (scoresT[k,q] = kT.T@qT),
the exp output feeds attn@v as the *moving* operand, and a ones column
appended to v yields the softmax row-sums as a 65th output row of the
same matmul.  Matmul operands bf16, fp32 PSUM accumulation.
"""

import numpy as np

import concourse.bacc as bacc
import concourse.mybir as mybir
import concourse.tile as tile
from concourse import bass_utils

F32 = mybir.dt.float32
BF16 = mybir.dt.bfloat16
EXP = mybir.ActivationFunctionType.Exp
MULT = mybir.AluOpType.mult

B, S, D, H = 2, 2048, 1024, 16
DH = D // H            # 64
NCORE = 8
GRP = 4                # cores per batch group
HL = H // GRP          # 4 local heads per core
DTILES = D // 128      # 8 contraction chunks
STILES = S // 128      # 16
QB = 128               # per-dest q block width
VW = DH + 1            # 65: v columns + ones column
VP = 128               # padded v block: [v(64) | ones(1) | zeros(63)]
NQ = 8                 # attention quarters (pr, h, qh)

_CACHE = {}


def _build():
    nc = bacc.Bacc("TRN2", target_bir_lowering=False, debug=False,
                   num_devices=NCORE)

    xT_d = nc.dram_tensor("xT", [D, S], BF16, kind="ExternalInput")
    wqk_d = nc.dram_tensor("wqk", [D, 2 * HL * DH], BF16, kind="ExternalInput")
    wv_d = nc.dram_tensor("wv", [D, HL * DH], BF16, kind="ExternalInput")
    bqk_d = nc.dram_tensor("bqk", [2 * HL * DH], F32, kind="ExternalInput")
    bv_d = nc.dram_tensor("bv", [HL * DH], F32, kind="ExternalInput")
    wout_d = nc.dram_tensor("wout", [D, D], BF16, kind="ExternalInput")
    bout_d = nc.dram_tensor("bout", [D], F32, kind="ExternalInput")
    ident_d = nc.dram_tensor("ident", [128, 128], BF16, kind="ExternalInput")
    out_d = nc.dram_tensor("out", [4 * QB, D], BF16, kind="ExternalOutput")

    groups = [list(range(NCORE))]

    with tile.TileContext(nc) as tc:
        with (
            tc.tile_pool(name="persist", bufs=1) as pers,
            tc.tile_pool(name="big", bufs=DTILES) as big,
            tc.tile_pool(name="wsmall", bufs=1) as wsmall,
            tc.tile_pool(name="ppool", bufs=8) as ppool,
            tc.tile_pool(name="npool", bufs=2) as npool,
            tc.tile_pool(name="fin", bufs=2) as fin,
            tc.tile_pool(name="dram", bufs=1, space="DRAM") as dram,
        ):
            # ---- persistent SBUF tensors ----
            kT = pers.tile([128, 2 * S], BF16, tag="kT")
            qp = pers.tile([128, 4 * S], BF16, tag="qp")
            vext = pers.tile([128, STILES * HL * VP], BF16, tag="vext")
            # aout: 32 col-blocks of 128: block b = pr*16 + 2*jj + qh,
            # rows h*64..h*64+64 filled by quarter (pr, h, qh)'s exchange.
            aout = pers.tile([128, 32 * QB], BF16, tag="aout")
            outacc = pers.tile([128, 4 * D], BF16, tag="outacc")
            bqk_sb = pers.tile([128, 4], F32, tag="bqk_sb")
            bv_sb = pers.tile([128, HL * DH], F32, tag="bv_sb")
            bout_bf = pers.tile([128, D], BF16, tag="bout_bf")
            bout_row = ppool.tile([1, D], F32, tag="P", name="bout_row")
            ident = pers.tile([128, 128], BF16, tag="ident")
            e0m = pers.tile([128, 128], BF16, tag="e0m")

            wqk_sb = wsmall.tile([128, DTILES * 512], BF16, tag="wqk_sb")
            wv_sb = wsmall.tile([128, DTILES * 256], BF16, tag="wv_sb")
            wout_sb = wsmall.tile([128, DTILES * D], BF16, tag="wout_sb")

            # ---- heater seed + all pure memsets first on DVE ----
            heat_b = wsmall.tile([128, 512], BF16, tag="heat_b")
            nc.vector.memset(heat_b[:], 0.25)
            nc.vector.memset(bout_bf[:], 0.0)
            nc.vector.memset(e0m[:], 0.0)
            nc.vector.memset(e0m[0:1, :], 1.0)
            nc.vector.memset(
                vext[:].rearrange("p (b w) -> p b w", w=VP)[:, :, DH + 1:VP],
                0.0)
            nc.vector.memset(
                vext[:].rearrange("p (b w) -> p b w", w=VP)[:, :, DH:DH + 1],
                1.0)
            for pr in range(2):
                nc.vector.memset(qp[64:128, (2 * pr) * S:(2 * pr + 1) * S], 0.0)
                nc.vector.memset(qp[0:64, (2 * pr + 1) * S:(2 * pr + 2) * S], 0.0)

            # ---- input DMAs.  The phase-A critical wqk/x stream issues
            # FIRST on both queues (each descriptor costs ~0.7us of issue
            # time on its engine); small transfers follow on sync (needed
            # only once phase A drains); wout (tail-only) last. ----
            engs = [nc.sync, nc.gpsimd]
            xt_tiles = [big.tile([128, S], BF16, tag="big", name=f"xt{dt}")
                        for dt in range(DTILES)]
            for dt in range(DTILES):
                q = engs[0] if dt < 4 else engs[1]
                q.dma_start(
                    wqk_sb[:, dt * 512:(dt + 1) * 512],
                    wqk_d[dt * 128:(dt + 1) * 128, :])
                q.dma_start(xt_tiles[dt][:], xT_d[dt * 128:(dt + 1) * 128, :])
            DT_ORDER = [0, 4, 1, 5, 2, 6, 3, 7]
            engs[1].dma_start(
                wv_sb[:].rearrange("p (d c) -> p d c", c=256),
                wv_d[:].rearrange("(d p) c -> p d c", p=128))
            # preload the gpsimd custom-op library (partition_broadcast's
            # ~15us LOAD_LIB) while the PE runs phase A; scratch target.
            lib_warm = wsmall.tile([64, 64], F32, tag="lib_warm")
            heat_f32 = wsmall.tile([1, 64], F32, tag="heat_f32")
            nc.vector.memset(heat_f32[:], 1.0)
            nc.gpsimd.partition_broadcast(lib_warm[:], heat_f32[:1, :])
            # small transfers: bqk/bv needed at phase-A drain (~25us),
            # bout/ident only at the tail.  bv_sb broadcast is done by the
            # DMA itself (0-stride source), not a gpsimd custom op.
            nc.sync.dma_start(bqk_sb[:],
                              bqk_d[:].rearrange("(e p) -> p e", p=128))
            nc.sync.dma_start(
                bv_sb[:], bv_d[:].unsqueeze(0).broadcast_to([128, HL * DH]))
            nc.sync.dma_start(bout_row[:], bout_d[:].unsqueeze(0))
            nc.sync.dma_start(ident[:], ident_d[:])
            engs[0].dma_start(
                wout_sb[:].rearrange("p (d c) -> p d c", c=D),
                wout_d[:].rearrange("(d p) c -> p d c", p=128))
            nc.vector.tensor_copy(bout_bf[0:1, :], bout_row[:1, :])

            # ---- pre-warm heater ----
            NHEAT = 40
            if NHEAT:
                with tc.tile_pool(name="psH", bufs=1, space="PSUM") as psH:
                    ph = psH.tile([128, 512], F32, tag="psH")
                    for i in range(NHEAT):
                        nc.tensor.matmul(ph[:, 0:256], heat_b[:, 0:128],
                                         heat_b[:, 0:256],
                                         start=True, stop=True)

            # ---- projection helpers ----
            def qk_store(et, sh, acc):
                dsts = slice(sh * 1024, (sh + 1) * 1024)
                if et >= 2:
                    pr = et - 2
                    nc.vector.tensor_scalar_add(
                        kT[:, pr * S:(pr + 1) * S][:, dsts],
                        acc[:], bqk_sb[:, et:et + 1])
                else:
                    pr = et
                    nc.vector.tensor_scalar_add(
                        qp[0:64, (2 * pr) * S:(2 * pr + 1) * S][:, dsts],
                        acc[0:64, :], bqk_sb[0:64, et:et + 1])
                    nc.vector.tensor_scalar_add(
                        qp[64:128, (2 * pr + 1) * S:(2 * pr + 2) * S][:, dsts],
                        acc[64:128, :], bqk_sb[64:128, et:et + 1])

            def heat(pool, tag, n, cols=512):
                # p-state keep-warm: dependency-free matmuls into a scratch
                # PSUM tile (fresh alloc so pool rotation stays shallow)
                hacc = pool.tile([128, 512], F32, tag=tag, name="hgap")
                for _ in range(n):
                    nc.tensor.matmul(hacc[:, 0:cols], heat_b[:, 0:128],
                                     heat_b[:, 0:cols], start=True, stop=True)

            def qk_chunk(pool, tag, et, sh, c):
                acc = pool.tile([128, 512], F32, tag=tag,
                                name=f"qk{et}{sh}{c}")
                sl = slice(sh * 1024 + c * 512, sh * 1024 + (c + 1) * 512)
                for dt in range(DTILES):
                    nc.tensor.matmul(
                        acc[:],
                        wqk_sb[:, dt * 512 + et * 128:
                               dt * 512 + (et + 1) * 128],
                        xt_tiles[dt][:, sl],
                        start=(dt == 0), stop=(dt == DTILES - 1))
                if et >= 2:
                    pr = et - 2
                    nc.vector.tensor_scalar_add(
                        kT[:, pr * S:(pr + 1) * S][:, sl],
                        acc[:], bqk_sb[:, et:et + 1])
                else:
                    pr = et
                    nc.vector.tensor_scalar_add(
                        qp[0:64, (2 * pr) * S:(2 * pr + 1) * S][:, sl],
                        acc[0:64, :], bqk_sb[0:64, et:et + 1])
                    nc.vector.tensor_scalar_add(
                        qp[64:128, (2 * pr + 1) * S:(2 * pr + 2) * S][:, sl],
                        acc[64:128, :], bqk_sb[64:128, et:et + 1])

            def v_acc(pool, tag, st):
                acc = pool.tile([128, HL * DH], F32, tag=tag, name=f"v{st}")
                for dt in range(DTILES):
                    nc.tensor.matmul(
                        acc[:],
                        xt_tiles[dt][:, st * 128:(st + 1) * 128],
                        wv_sb[:, dt * 256:(dt + 1) * 256],
                        start=(dt == 0), stop=(dt == DTILES - 1))
                base = st * HL * VP
                vv = vext[:, base:base + HL * VP].rearrange(
                    "p (h w) -> p h w", h=HL)
                nc.vector.tensor_add(
                    vv[:, :, 0:DH],
                    acc[:].rearrange("p (h w) -> p h w", h=HL),
                    bv_sb[:].rearrange("p (h w) -> p h w", h=HL))

            # ---- phase A: k pair 0 (both halves) + q pair 0 first half,
            # 3 live accumulators consumed per-dt as DMAs land; heat
            # trickles between dt groups keep the p-state up through the
            # DMA-paced ramp.  (et0, sh1) is paced into quarter 0 as two
            # qk_chunk fillers.  Drains ordered so quarter 0's needs
            # (k-sh0, q-sh0) clear the DVE first. ----
            PHA = [(2, 0), (0, 0), (2, 1)]
            with (
                tc.tile_pool(name="psA0", bufs=3, space="PSUM") as psA0,
                tc.tile_pool(name="psH2", bufs=1, space="PSUM") as psH2,
            ):
                accs = {k: psA0.tile([128, 1024], F32, tag="psA0",
                                     name=f"qa{k[0]}{k[1]}")
                        for k in PHA}
                hacc = psH2.tile([128, 512], F32, tag="psH2", name="hacc")
                for i, dt in enumerate(DT_ORDER):
                    last = i == DTILES - 1
                    for et, sh in PHA:
                        for c in range(2):
                            sl = slice(sh * 1024 + c * 512,
                                       sh * 1024 + (c + 1) * 512)
                            nc.tensor.matmul(
                                accs[(et, sh)][:, c * 512:(c + 1) * 512],
                                wqk_sb[:, dt * 512 + et * 128:
                                       dt * 512 + (et + 1) * 128],
                                xt_tiles[dt][:, sl],
                                start=(i == 0), stop=last)
                        if last:
                            qk_store(et, sh, accs[(et, sh)])
                    if not last:
                        for _ in range(5):
                            nc.tensor.matmul(hacc[:], heat_b[:, 0:128],
                                             heat_b[:], start=True, stop=True)

            wout_tiles = [wout_sb[:, ec * D:(ec + 1) * D]
                          for ec in range(DTILES)]

            # ---- attention: 8 quarters (pr, h, qh), one AllToAll each ----
            a2a_in = [dram.tile([NCORE, 64, QB], BF16, tag=f"a2a_in{q}",
                                name=f"a2a_in{q}") for q in range(NQ)]
            a2a_out = [dram.tile([NCORE, 64, QB], BF16, tag=f"a2a_out{q}",
                                 name=f"a2a_out{q}") for q in range(NQ)]

            def aout_land(qi):
                # pull exchange qi's result into aout (rows h*64..,
                # col blocks pr*16 + 2*jj + qh for jj in 0..7)
                pr, h, qh = qi >> 2, (qi >> 1) & 1, qi & 1
                dst = aout[h * 64:(h + 1) * 64, :].rearrange(
                    "p (a j q c) -> p a j q c", a=2, j=NCORE, q=2)[:, pr, :, qh, :]
                nc.sync.dma_start(
                    dst, a2a_out[qi][:].rearrange("d p c -> p d c"))

            # psS: 3 slots so filler allocations don't collapse the
            # scores->exp double-buffering; po single-buffered (outproj
            # accumulators borrow psS slots at the tail).
            with (
                tc.tile_pool(name="psS", bufs=3, space="PSUM") as psS,
                tc.tile_pool(name="psO", bufs=1, space="PSUM") as psO,
            ):
                # v tiles 0,1 ahead of the loop
                for st in range(2):
                    v_acc(psS, "psS", st)

                def outproj_block(p, gb, qb):
                    acc = psS.tile([128, D], F32, tag="psS",
                                   name=f"op{gb}{qb}")
                    oa = outacc[:, (gb * 2 + qb) * D:(gb * 2 + qb + 1) * D]
                    for c in range(2):
                        nc.tensor.matmul(
                            acc[:, c * 512:(c + 1) * 512],
                            e0m[:] if p == 0 else ident[:],
                            (bout_bf if p == 0 else oa)
                            [:, c * 512:(c + 1) * 512],
                            start=True, stop=False)
                    for jr in range(GRP):
                        jj = gb * GRP + jr
                        blk = p * 16 + 2 * jj + qb
                        for c in range(2):
                            nc.tensor.matmul(
                                acc[:, c * 512:(c + 1) * 512],
                                aout[:, blk * QB:(blk + 1) * QB],
                                wout_tiles[p * GRP + jr][:, c * 512:(c + 1) * 512],
                                start=False, stop=(jr == GRP - 1))
                    if p == 0:
                        nc.scalar.copy(oa, acc[:])
                    else:
                        res = fin.tile([128, D], BF16, tag="res")
                        nc.scalar.copy(res[:], acc[:])
                        row = (gb * 2 + qb) * QB
                        (nc.sync if gb == 0 else nc.gpsimd).dma_start(
                            out_d[row:row + QB, :], res[:])

                # filler schedule: quarter 0 -> v tiles 2..15 at kt 0..13
                # and the deferred (et0, sh1) q-projection at kt 14..15;
                # quarters 1,2 -> pair-1 qk chunks at kt 1,5,9,13;
                # quarter 7 -> outproj pair-0 bursts at kt 2,6,10,14.
                # Every other iteration gets one 512-col heat matmul so the
                # PE never idles waiting on the exp stream (idle drops the
                # p-state and with it the matmul clock).
                qkc = [(et, sh, c) for et in (3, 1) for sh in range(2)
                       for c in range(2)]

                for qi in range(NQ):
                    pr, h, qh = qi >> 2, (qi >> 1) & 1, qi & 1
                    lh = 2 * pr + h
                    po = psO.tile([128, 1024], F32, tag="psO", name="po")
                    prev_p = None
                    for kt in range(STILES):
                        ps = psS.tile([128, 1024], F32, tag="psS", name="ps")
                        for c in range(2):
                            nc.tensor.matmul(
                                ps[:, c * 512:(c + 1) * 512],
                                kT[:, pr * S + kt * 128:
                                   pr * S + (kt + 1) * 128],
                                qp[:, lh * S + qh * 1024 + c * 512:
                                   lh * S + qh * 1024 + (c + 1) * 512],
                                start=True, stop=True)
                        if prev_p is not None:
                            vb = ((kt - 1) * HL + lh) * VP
                            for c in range(2):
                                nc.tensor.matmul(
                                    po[:, c * 512:(c + 1) * 512],
                                    vext[:, vb:vb + VP],
                                    prev_p[:, c * 512:(c + 1) * 512],
                                    start=(kt - 1 == 0), stop=False)
                        pexp = ppool.tile([128, 1024], BF16, tag="P",
                                          name="pexp")
                        nc.scalar.activation(pexp[:], ps[:], EXP, scale=0.125)
                        prev_p = pexp
                        # fillers
                        if qi == 0 and kt < 14:
                            v_acc(psS, "psS", kt + 2)
                        elif qi == 0:
                            qk_chunk(psS, "psS", 0, 1, kt - 14)
                        elif qi in (1, 2) and kt % 4 == 1:
                            et, sh, c = qkc[(qi - 1) * 4 + kt // 4]
                            qk_chunk(psS, "psS", et, sh, c)
                        elif qi == NQ - 1 and kt % 4 == 2:
                            i = kt // 4
                            outproj_block(0, i // 2, i % 2)
                    vb = ((STILES - 1) * HL + lh) * VP
                    for c in range(2):
                        nc.tensor.matmul(
                            po[:, c * 512:(c + 1) * 512],
                            vext[:, vb:vb + VP],
                            prev_p[:, c * 512:(c + 1) * 512],
                            start=False, stop=True)

                    last_q = qi == NQ - 1

                    # normalize quarter -> attn [64, 1024] bf16
                    rs_row = npool.tile([1, 1024], F32, tag="rs_row",
                                        name="rs_row")
                    rs_rec = npool.tile([1, 1024], F32, tag="rs_rec",
                                        name="rs_rec")
                    rs_b = npool.tile([64, 1024], F32, tag="rs_b",
                                      name="rs_b")
                    attn = npool.tile([64, 1024], BF16, tag="attn",
                                      name="attn")
                    nc.vector.tensor_copy(rs_row[:], po[DH:VW, :])
                    nc.vector.reciprocal_approx_fast(rs_rec[:], rs_row[:1, :])
                    nc.gpsimd.partition_broadcast(rs_b[:], rs_rec[:1, :])
                    if last_q:
                        nc.vector.tensor_tensor(attn[:], po[0:DH, :],
                                                rs_b[:], MULT)
                    else:
                        stg = npool.tile([DH, 1024], F32, tag="stg",
                                         name="stg")
                        nc.vector.tensor_copy(stg[:], po[0:DH, :])
                        nc.vector.tensor_tensor(attn[:], stg[:, :],
                                                rs_b[:], MULT)
                    # exchange input: one descriptor, 8 dest slices of 128
                    nc.sync.dma_start(
                        a2a_in[qi][:].rearrange("d p c -> p d c"),
                        attn[:].rearrange("p (d c) -> p d c", c=QB))
                    nc.gpsimd.collective_compute(
                        "AllToAll", mybir.AluOpType.bypass,
                        replica_groups=groups,
                        ins=[a2a_in[qi][:].opt()],
                        outs=[a2a_out[qi][:].opt()])
                    # land results late enough that the sync queue never
                    # blocks on an in-flight collective (the CC stream can
                    # back up ~40-100us behind the init barrier): pair-0
                    # at quarters 5/6 (needed by the quarter-7 outproj
                    # bursts), pair-1 at the tail.
                    if qi == 5:
                        aout_land(0), aout_land(1)
                    elif qi == 6:
                        aout_land(2), aout_land(3), aout_land(4)
                    elif qi == 7:
                        aout_land(5)
                        heat(psS, "psS", 35)
                        aout_land(6)
                        aout_land(7)

                for gb in range(2):
                    for qb in range(2):
                        outproj_block(1, gb, qb)

    nc.compile()
    return nc


def _shard(inputs):
    import ml_dtypes
    bf = ml_dtypes.bfloat16
    x = np.asarray(inputs["x"], np.float32)
    w_qkv = np.asarray(inputs["w_qkv"], np.float32)
    b_qkv = np.asarray(inputs["b_qkv"], np.float32)
    w_out = np.asarray(inputs["w_out"], np.float32)
    b_out = np.asarray(inputs["b_out"], np.float32)

    # wout rows permuted to match aout row order: for pair p, peer
    # rank-in-group jr, t in (0,1): head 4*jr + 2*p + t
    rows = []
    for p in (0, 1):
        for jr in range(GRP):
            for t in (0, 1):
                h = 4 * jr + 2 * p + t
                rows.append(w_out[h * DH:(h + 1) * DH, :])
    wout_perm = np.ascontiguousarray(np.concatenate(rows, 0))

    in_maps = []
    for c in range(NCORE):
        g, j = c // GRP, c % GRP
        cs = slice(j * HL * DH, (j + 1) * HL * DH)
        wqk = np.concatenate([w_qkv[:, :D][:, cs], w_qkv[:, D:2 * D][:, cs]], 1)
        bqk = np.concatenate([b_qkv[:D][cs], b_qkv[D:2 * D][cs]])
        in_maps.append({
            "xT": np.ascontiguousarray(x[g].T).astype(bf),
            "wqk": np.ascontiguousarray(wqk).astype(bf),
            "wv": np.ascontiguousarray(w_qkv[:, 2 * D:][:, cs]).astype(bf),
            "bqk": np.ascontiguousarray(bqk),
            "bv": np.ascontiguousarray(b_qkv[2 * D:][cs]),
            "wout": wout_perm.astype(bf),
            "bout": b_out,
            "ident": np.eye(128, dtype=np.float32).astype(bf),
        })
    return in_maps


def _install_ntff_hook():
    """The agent image's antenv lacks axon_hooks; shim it and register the
    ctypes NTFF profiler from trn_agent_boot so trace=True works."""
    import sys
    import types

    if "antenv.axon_hooks" in sys.modules:
        return
    import antenv

    mod = types.ModuleType("antenv.axon_hooks")
    mod._hook = None
    mod.set_axon_ntff_profile_hook = lambda h: setattr(mod, "_hook", h)
    mod.get_axon_ntff_profile_hook = lambda: mod._hook
    sys.modules["antenv.axon_hooks"] = mod
    antenv.axon_hooks = mod
    try:
        from trn_agent_boot.trn_boot import _ntff_profile_via_ctypes
        mod._hook = _ntff_profile_via_ctypes("/opt/axon/libaxon_pjrt.so")
    except Exception as e:
        print(f"ntff hook install failed: {e}")


def _run(inputs, trace=False):
    if trace:
        _install_ntff_hook()
    if "nc" not in _CACHE:
        _CACHE["nc"] = _build()
    nc = _CACHE["nc"]
    in_maps = _shard(inputs)
    r = bass_utils.run_bass_kernel_spmd(
        nc, in_maps, core_ids=list(range(NCORE)), trace=trace)
    out = np.empty((B, S, D), np.float32)
    for c in range(NCORE):
        for g in range(B):
            for qh in range(2):
                out[g, qh * 1024 + c * QB: qh * 1024 + (c + 1) * QB, :] = \
                    r.results[c]["out"][(g * 2 + qh) * QB:
                                        (g * 2 + qh + 1) * QB].astype(np.float32)
    return out, r


def kernel(**inputs) -> np.ndarray:
    out, _ = _run(inputs, trace=False)
    return out


# revision 11
# speedup vs baseline: 1.0837x; 1.0050x over previous
"""Distributed multi-head attention kernel for Trainium2 (8 NeuronCores).

Problem: B=2, S=2048, D=1024, H=16 heads, DH=64.
  qkv = x @ w_qkv + b_qkv ; per-head softmax(q k^T / 8) v ; out proj.

Sharding: core c = g*4 + j handles batch g and heads 4j..4j+3.

Stage-B schedule:
  - Head-outer attention: for (pair, head, q-half) process 16 k-tiles;
    each quarter's [64, 1024] attention output is normalized and
    AllToAll'd immediately as a [8, 64, 128] exchange (128 KiB), so 7
    of the 8 collectives fully overlap attention and only the last
    ~8us is exposed (vs. one 512 KiB collective at the tail).
  - Dest mapping: core c owns q rows {qh*1024 + c*128} of each batch,
    so every quarter's exchange carries data for all 8 peers.
  - Fillers keep the PE ahead of the ACT exp stream: v tiles 2-15
    paced into quarter 0, the pair-1 q/k projection into quarters 1-2.
  - aout bounce DMAs are batched two quarters behind their collective
    so the sync queue never blocks on an in-flight collective.
  - Output staged and DMA'd as bf16 (host upcasts).

Layout trick: scores are computed transposed (scoresT[k,q] = kT.T@qT),
the exp output feeds attn@v as the *moving* operand, and a ones column
appended to v yields the softmax row-sums as a 65th output row of the
same matmul.  Matmul operands bf16, fp32 PSUM accumulation.
"""

import numpy as np

import concourse.bacc as bacc
import concourse.mybir as mybir
import concourse.tile as tile
from concourse import bass_utils

F32 = mybir.dt.float32
BF16 = mybir.dt.bfloat16
EXP = mybir.ActivationFunctionType.Exp
MULT = mybir.AluOpType.mult

B, S, D, H = 2, 2048, 1024, 16
DH = D // H            # 64
NCORE = 8
GRP = 4                # cores per batch group
HL = H // GRP          # 4 local heads per core
DTILES = D // 128      # 8 contraction chunks
STILES = S // 128      # 16
QB = 128               # per-dest q block width
VW = DH + 1            # 65: v columns + ones column
VP = 128               # padded v block: [v(64) | ones(1) | zeros(63)]
NQ = 8                 # attention quarters (pr, h, qh)

_CACHE = {}


def _build():
    nc = bacc.Bacc("TRN2", target_bir_lowering=False, debug=False,
                   num_devices=NCORE)

    xT_d = nc.dram_tensor("xT", [D, S], BF16, kind="ExternalInput")
    wqk_d = nc.dram_tensor("wqk", [D, 2 * HL * DH], BF16, kind="ExternalInput")
    wv_d = nc.dram_tensor("wv", [D, HL * DH], BF16, kind="ExternalInput")
    bqk_d = nc.dram_tensor("bqk", [2 * HL * DH], F32, kind="ExternalInput")
    bv_d = nc.dram_tensor("bv", [HL * DH], F32, kind="ExternalInput")
    wout_d = nc.dram_tensor("wout", [D, D], BF16, kind="ExternalInput")
    bout_d = nc.dram_tensor("bout", [D], F32, kind="ExternalInput")
    ident_d = nc.dram_tensor("ident", [128, 128], BF16, kind="ExternalInput")
    out_d = nc.dram_tensor("out", [4 * QB, D], BF16, kind="ExternalOutput")

    groups = [list(range(NCORE))]

    with tile.TileContext(nc) as tc:
        with (
            tc.tile_pool(name="persist", bufs=1) as pers,
            tc.tile_pool(name="big", bufs=DTILES) as big,
            tc.tile_pool(name="wsmall", bufs=1) as wsmall,
            tc.tile_pool(name="ppool", bufs=8) as ppool,
            tc.tile_pool(name="npool", bufs=2) as npool,
            tc.tile_pool(name="fin", bufs=2) as fin,
            tc.tile_pool(name="dram", bufs=1, space="DRAM") as dram,
        ):
            # ---- persistent SBUF tensors ----
            kT = pers.tile([128, 2 * S], BF16, tag="kT")
            qp = pers.tile([128, 4 * S], BF16, tag="qp")
            vext = pers.tile([128, STILES * HL * VP], BF16, tag="vext")
            # aout: 32 col-blocks of 128: block b = pr*16 + 2*jj + qh,
            # rows h*64..h*64+64 filled by quarter (pr, h, qh)'s exchange.
            aout = pers.tile([128, 32 * QB], BF16, tag="aout")
            outacc = pers.tile([128, 4 * D], BF16, tag="outacc")
            bqk_sb = pers.tile([128, 4], F32, tag="bqk_sb")
            bv_sb = pers.tile([128, HL * DH], F32, tag="bv_sb")
            bout_bf = pers.tile([128, D], BF16, tag="bout_bf")
            bout_row = ppool.tile([1, D], F32, tag="P", name="bout_row")
            ident = pers.tile([128, 128], BF16, tag="ident")
            e0m = pers.tile([128, 128], BF16, tag="e0m")

            wqk_sb = wsmall.tile([128, DTILES * 512], BF16, tag="wqk_sb")
            wv_sb = wsmall.tile([128, DTILES * 256], BF16, tag="wv_sb")
            wout_sb = wsmall.tile([128, DTILES * D], BF16, tag="wout_sb")

            # ---- heater seed + all pure memsets first on DVE.  The heater
            # data is random bits: the clock governor responds to POWER
            # draw, and constant-data matmuls (no wire switching) don't
            # ramp it -- measured 109ns/256col for constant heats vs 268ns
            # throttled steady-state for real-data matmuls. ----
            heat_b = wsmall.tile([128, 512], BF16, tag="heat_b")
            nc.vector.random(heat_b[:])
            nc.vector.memset(bout_bf[:], 0.0)
            nc.vector.memset(e0m[:], 0.0)
            nc.vector.memset(e0m[0:1, :], 1.0)
            nc.vector.memset(
                vext[:].rearrange("p (b w) -> p b w", w=VP)[:, :, DH + 1:VP],
                0.0)
            nc.vector.memset(
                vext[:].rearrange("p (b w) -> p b w", w=VP)[:, :, DH:DH + 1],
                1.0)
            for pr in range(2):
                nc.vector.memset(qp[64:128, (2 * pr) * S:(2 * pr + 1) * S], 0.0)
                nc.vector.memset(qp[0:64, (2 * pr + 1) * S:(2 * pr + 2) * S], 0.0)

            # ---- input DMAs.  The phase-A critical wqk/x stream issues
            # FIRST on both queues (each descriptor costs ~0.7us of issue
            # time on its engine); small transfers follow on sync (needed
            # only once phase A drains); wout (tail-only) last. ----
            engs = [nc.sync, nc.gpsimd]
            xt_tiles = [big.tile([128, S], BF16, tag="big", name=f"xt{dt}")
                        for dt in range(DTILES)]
            for dt in range(DTILES):
                q = engs[0] if dt < 4 else engs[1]
                q.dma_start(
                    wqk_sb[:, dt * 512:(dt + 1) * 512],
                    wqk_d[dt * 128:(dt + 1) * 128, :])
                q.dma_start(xt_tiles[dt][:], xT_d[dt * 128:(dt + 1) * 128, :])
            DT_ORDER = [0, 4, 1, 5, 2, 6, 3, 7]
            engs[1].dma_start(
                wv_sb[:].rearrange("p (d c) -> p d c", c=256),
                wv_d[:].rearrange("(d p) c -> p d c", p=128))
            # preload the gpsimd custom-op library (partition_broadcast's
            # ~15us LOAD_LIB) while the PE runs phase A; scratch target.
            lib_warm = wsmall.tile([64, 64], F32, tag="lib_warm")
            heat_f32 = wsmall.tile([1, 64], F32, tag="heat_f32")
            nc.vector.memset(heat_f32[:], 1.0)
            nc.gpsimd.partition_broadcast(lib_warm[:], heat_f32[:1, :])
            # small transfers: bqk/bv needed at phase-A drain (~25us),
            # bout/ident only at the tail.  bv_sb broadcast is done by the
            # DMA itself (0-stride source), not a gpsimd custom op.
            nc.sync.dma_start(bqk_sb[:],
                              bqk_d[:].rearrange("(e p) -> p e", p=128))
            nc.sync.dma_start(
                bv_sb[:], bv_d[:].unsqueeze(0).broadcast_to([128, HL * DH]))
            nc.sync.dma_start(bout_row[:], bout_d[:].unsqueeze(0))
            nc.sync.dma_start(ident[:], ident_d[:])
            engs[0].dma_start(
                wout_sb[:].rearrange("p (d c) -> p d c", c=D),
                wout_d[:].rearrange("(d p) c -> p d c", p=128))
            nc.vector.tensor_copy(bout_bf[0:1, :], bout_row[:1, :])

            # ---- pre-warm heater ----
            NHEAT = 28
            if NHEAT:
                with tc.tile_pool(name="psH", bufs=1, space="PSUM") as psH:
                    ph = psH.tile([128, 512], F32, tag="psH")
                    for i in range(NHEAT):
                        nc.tensor.matmul(ph[:, 0:256], heat_b[:, 0:128],
                                         heat_b[:, 0:256],
                                         start=True, stop=True)

            # ---- projection helpers ----
            def qk_store(et, sh, acc):
                dsts = slice(sh * 1024, (sh + 1) * 1024)
                if et >= 2:
                    pr = et - 2
                    nc.vector.tensor_scalar_add(
                        kT[:, pr * S:(pr + 1) * S][:, dsts],
                        acc[:], bqk_sb[:, et:et + 1])
                else:
                    pr = et
                    nc.vector.tensor_scalar_add(
                        qp[0:64, (2 * pr) * S:(2 * pr + 1) * S][:, dsts],
                        acc[0:64, :], bqk_sb[0:64, et:et + 1])
                    nc.vector.tensor_scalar_add(
                        qp[64:128, (2 * pr + 1) * S:(2 * pr + 2) * S][:, dsts],
                        acc[64:128, :], bqk_sb[64:128, et:et + 1])

            def heat(pool, tag, n, cols=512):
                # p-state keep-warm: dependency-free matmuls into a scratch
                # PSUM tile (fresh alloc so pool rotation stays shallow)
                hacc = pool.tile([128, 512], F32, tag=tag, name="hgap")
                for _ in range(n):
                    nc.tensor.matmul(hacc[:, 0:cols], heat_b[:, 0:128],
                                     heat_b[:, 0:cols], start=True, stop=True)

            def qk_chunk(pool, tag, et, sh, c):
                acc = pool.tile([128, 512], F32, tag=tag,
                                name=f"qk{et}{sh}{c}")
                sl = slice(sh * 1024 + c * 512, sh * 1024 + (c + 1) * 512)
                for dt in range(DTILES):
                    nc.tensor.matmul(
                        acc[:],
                        wqk_sb[:, dt * 512 + et * 128:
                               dt * 512 + (et + 1) * 128],
                        xt_tiles[dt][:, sl],
                        start=(dt == 0), stop=(dt == DTILES - 1))
                if et >= 2:
                    pr = et - 2
                    nc.vector.tensor_scalar_add(
                        kT[:, pr * S:(pr + 1) * S][:, sl],
                        acc[:], bqk_sb[:, et:et + 1])
                else:
                    pr = et
                    nc.vector.tensor_scalar_add(
                        qp[0:64, (2 * pr) * S:(2 * pr + 1) * S][:, sl],
                        acc[0:64, :], bqk_sb[0:64, et:et + 1])
                    nc.vector.tensor_scalar_add(
                        qp[64:128, (2 * pr + 1) * S:(2 * pr + 2) * S][:, sl],
                        acc[64:128, :], bqk_sb[64:128, et:et + 1])

            def v_acc(pool, tag, st):
                acc = pool.tile([128, HL * DH], F32, tag=tag, name=f"v{st}")
                for dt in range(DTILES):
                    nc.tensor.matmul(
                        acc[:],
                        xt_tiles[dt][:, st * 128:(st + 1) * 128],
                        wv_sb[:, dt * 256:(dt + 1) * 256],
                        start=(dt == 0), stop=(dt == DTILES - 1))
                base = st * HL * VP
                vv = vext[:, base:base + HL * VP].rearrange(
                    "p (h w) -> p h w", h=HL)
                nc.vector.tensor_add(
                    vv[:, :, 0:DH],
                    acc[:].rearrange("p (h w) -> p h w", h=HL),
                    bv_sb[:].rearrange("p (h w) -> p h w", h=HL))

            # ---- phase A: k pair 0 (both halves) + q pair 0 first half,
            # 3 live accumulators consumed per-dt as DMAs land; heat
            # trickles between dt groups keep the p-state up through the
            # DMA-paced ramp.  (et0, sh1) is paced into quarter 0 as two
            # qk_chunk fillers.  Drains ordered so quarter 0's needs
            # (k-sh0, q-sh0) clear the DVE first. ----
            PHA = [(2, 0), (0, 0), (2, 1)]
            with (
                tc.tile_pool(name="psA0", bufs=3, space="PSUM") as psA0,
                tc.tile_pool(name="psH2", bufs=1, space="PSUM") as psH2,
            ):
                accs = {k: psA0.tile([128, 1024], F32, tag="psA0",
                                     name=f"qa{k[0]}{k[1]}")
                        for k in PHA}
                hacc = psH2.tile([128, 512], F32, tag="psH2", name="hacc")
                for i, dt in enumerate(DT_ORDER):
                    last = i == DTILES - 1
                    for et, sh in PHA:
                        for c in range(2):
                            sl = slice(sh * 1024 + c * 512,
                                       sh * 1024 + (c + 1) * 512)
                            nc.tensor.matmul(
                                accs[(et, sh)][:, c * 512:(c + 1) * 512],
                                wqk_sb[:, dt * 512 + et * 128:
                                       dt * 512 + (et + 1) * 128],
                                xt_tiles[dt][:, sl],
                                start=(i == 0), stop=last)
                        if last:
                            qk_store(et, sh, accs[(et, sh)])
                    if not last:
                        for _ in range(5):
                            nc.tensor.matmul(hacc[:], heat_b[:, 0:128],
                                             heat_b[:], start=True, stop=True)

            wout_tiles = [wout_sb[:, ec * D:(ec + 1) * D]
                          for ec in range(DTILES)]

            # ---- attention: 8 quarters (pr, h, qh), one AllToAll each ----
            a2a_in = [dram.tile([NCORE, 64, QB], BF16, tag=f"a2a_in{q}",
                                name=f"a2a_in{q}") for q in range(NQ)]
            a2a_out = [dram.tile([NCORE, 64, QB], BF16, tag=f"a2a_out{q}",
                                 name=f"a2a_out{q}") for q in range(NQ)]

            def aout_land(qi):
                # pull exchange qi's result into aout (rows h*64..,
                # col blocks pr*16 + 2*jj + qh for jj in 0..7)
                pr, h, qh = qi >> 2, (qi >> 1) & 1, qi & 1
                dst = aout[h * 64:(h + 1) * 64, :].rearrange(
                    "p (a j q c) -> p a j q c", a=2, j=NCORE, q=2)[:, pr, :, qh, :]
                nc.sync.dma_start(
                    dst, a2a_out[qi][:].rearrange("d p c -> p d c"))

            # psS: 3 slots so filler allocations don't collapse the
            # scores->exp double-buffering; po single-buffered (outproj
            # accumulators borrow psS slots at the tail).
            with (
                tc.tile_pool(name="psS", bufs=3, space="PSUM") as psS,
                tc.tile_pool(name="psO", bufs=1, space="PSUM") as psO,
            ):
                # v tiles 0,1 ahead of the loop
                for st in range(2):
                    v_acc(psS, "psS", st)

                def outproj_block(p, gb, qb):
                    acc = psS.tile([128, D], F32, tag="psS",
                                   name=f"op{gb}{qb}")
                    oa = outacc[:, (gb * 2 + qb) * D:(gb * 2 + qb + 1) * D]
                    for c in range(2):
                        nc.tensor.matmul(
                            acc[:, c * 512:(c + 1) * 512],
                            e0m[:] if p == 0 else ident[:],
                            (bout_bf if p == 0 else oa)
                            [:, c * 512:(c + 1) * 512],
                            start=True, stop=False)
                    for jr in range(GRP):
                        jj = gb * GRP + jr
                        blk = p * 16 + 2 * jj + qb
                        for c in range(2):
                            nc.tensor.matmul(
                                acc[:, c * 512:(c + 1) * 512],
                                aout[:, blk * QB:(blk + 1) * QB],
                                wout_tiles[p * GRP + jr][:, c * 512:(c + 1) * 512],
                                start=False, stop=(jr == GRP - 1))
                    if p == 0:
                        nc.scalar.copy(oa, acc[:])
                    else:
                        res = fin.tile([128, D], BF16, tag="res")
                        nc.scalar.copy(res[:], acc[:])
                        row = (gb * 2 + qb) * QB
                        (nc.sync if gb == 0 else nc.gpsimd).dma_start(
                            out_d[row:row + QB, :], res[:])

                # filler schedule: quarter 0 -> v tiles 2..15 at kt 0..13
                # and the deferred (et0, sh1) q-projection at kt 14..15;
                # quarters 1,2 -> pair-1 qk chunks at kt 1,5,9,13;
                # quarter 7 -> outproj pair-0 bursts at kt 2,6,10,14.
                # Every other iteration gets one 512-col heat matmul so the
                # PE never idles waiting on the exp stream (idle drops the
                # p-state and with it the matmul clock).
                qkc = [(et, sh, c) for et in (3, 1) for sh in range(2)
                       for c in range(2)]

                for qi in range(NQ):
                    pr, h, qh = qi >> 2, (qi >> 1) & 1, qi & 1
                    lh = 2 * pr + h
                    po = psO.tile([128, 1024], F32, tag="psO", name="po")
                    prev_p = None
                    for kt in range(STILES):
                        ps = psS.tile([128, 1024], F32, tag="psS", name="ps")
                        for c in range(2):
                            nc.tensor.matmul(
                                ps[:, c * 512:(c + 1) * 512],
                                kT[:, pr * S + kt * 128:
                                   pr * S + (kt + 1) * 128],
                                qp[:, lh * S + qh * 1024 + c * 512:
                                   lh * S + qh * 1024 + (c + 1) * 512],
                                start=True, stop=True)
                        if prev_p is not None:
                            vb = ((kt - 1) * HL + lh) * VP
                            for c in range(2):
                                nc.tensor.matmul(
                                    po[:, c * 512:(c + 1) * 512],
                                    vext[:, vb:vb + VP],
                                    prev_p[:, c * 512:(c + 1) * 512],
                                    start=(kt - 1 == 0), stop=False)
                        pexp = ppool.tile([128, 1024], BF16, tag="P",
                                          name="pexp")
                        nc.scalar.activation(pexp[:], ps[:], EXP, scale=0.125)
                        prev_p = pexp
                        # fillers
                        if qi == 0 and kt < 14:
                            v_acc(psS, "psS", kt + 2)
                        elif qi == 0:
                            qk_chunk(psS, "psS", 0, 1, kt - 14)
                        elif qi in (1, 2) and kt % 4 == 1:
                            et, sh, c = qkc[(qi - 1) * 4 + kt // 4]
                            qk_chunk(psS, "psS", et, sh, c)
                        elif qi == NQ - 1 and kt % 4 == 2:
                            i = kt // 4
                            outproj_block(0, i // 2, i % 2)
                    vb = ((STILES - 1) * HL + lh) * VP
                    for c in range(2):
                        nc.tensor.matmul(
                            po[:, c * 512:(c + 1) * 512],
                            vext[:, vb:vb + VP],
                            prev_p[:, c * 512:(c + 1) * 512],
                            start=False, stop=True)

                    last_q = qi == NQ - 1

                    # normalize quarter -> attn [64, 1024] bf16
                    rs_row = npool.tile([1, 1024], F32, tag="rs_row",
                                        name="rs_row")
                    rs_rec = npool.tile([1, 1024], F32, tag="rs_rec",
                                        name="rs_rec")
                    rs_b = npool.tile([64, 1024], F32, tag="rs_b",
                                      name="rs_b")
                    attn = npool.tile([64, 1024], BF16, tag="attn",
                                      name="attn")
                    nc.vector.tensor_copy(rs_row[:], po[DH:VW, :])
                    nc.vector.reciprocal_approx_fast(rs_rec[:], rs_row[:1, :])
                    nc.gpsimd.partition_broadcast(rs_b[:], rs_rec[:1, :])
                    if last_q:
                        nc.vector.tensor_tensor(attn[:], po[0:DH, :],
                                                rs_b[:], MULT)
                    else:
                        stg = npool.tile([DH, 1024], F32, tag="stg",
                                         name="stg")
                        nc.vector.tensor_copy(stg[:], po[0:DH, :])
                        nc.vector.tensor_tensor(attn[:], stg[:, :],
                                                rs_b[:], MULT)
                    # exchange input: one descriptor, 8 dest slices of 128
                    nc.sync.dma_start(
                        a2a_in[qi][:].rearrange("d p c -> p d c"),
                        attn[:].rearrange("p (d c) -> p d c", c=QB))
                    nc.gpsimd.collective_compute(
                        "AllToAll", mybir.AluOpType.bypass,
                        replica_groups=groups,
                        ins=[a2a_in[qi][:].opt()],
                        outs=[a2a_out[qi][:].opt()])
                    # land results late enough that the sync queue never
                    # blocks on an in-flight collective (the CC stream can
                    # back up ~40-100us behind the init barrier): pair-0
                    # at quarters 5/6 (needed by the quarter-7 outproj
                    # bursts), pair-1 at the tail.
                    if qi == 5:
                        aout_land(0), aout_land(1)
                    elif qi == 6:
                        aout_land(2), aout_land(3), aout_land(4)
                    elif qi == 7:
                        aout_land(5)
                        heat(psS, "psS", 35)
                        aout_land(6)
                        aout_land(7)

                for gb in range(2):
                    for qb in range(2):
                        outproj_block(1, gb, qb)

    nc.compile()
    return nc


def _shard(inputs):
    import ml_dtypes
    bf = ml_dtypes.bfloat16
    x = np.asarray(inputs["x"], np.float32)
    w_qkv = np.asarray(inputs["w_qkv"], np.float32)
    b_qkv = np.asarray(inputs["b_qkv"], np.float32)
    w_out = np.asarray(inputs["w_out"], np.float32)
    b_out = np.asarray(inputs["b_out"], np.float32)

    # wout rows permuted to match aout row order: for pair p, peer
    # rank-in-group jr, t in (0,1): head 4*jr + 2*p + t
    rows = []
    for p in (0, 1):
        for jr in range(GRP):
            for t in (0, 1):
                h = 4 * jr + 2 * p + t
                rows.append(w_out[h * DH:(h + 1) * DH, :])
    wout_perm = np.ascontiguousarray(np.concatenate(rows, 0))

    in_maps = []
    for c in range(NCORE):
        g, j = c // GRP, c % GRP
        cs = slice(j * HL * DH, (j + 1) * HL * DH)
        wqk = np.concatenate([w_qkv[:, :D][:, cs], w_qkv[:, D:2 * D][:, cs]], 1)
        bqk = np.concatenate([b_qkv[:D][cs], b_qkv[D:2 * D][cs]])
        in_maps.append({
            "xT": np.ascontiguousarray(x[g].T).astype(bf),
            "wqk": np.ascontiguousarray(wqk).astype(bf),
            "wv": np.ascontiguousarray(w_qkv[:, 2 * D:][:, cs]).astype(bf),
            "bqk": np.ascontiguousarray(bqk),
            "bv": np.ascontiguousarray(b_qkv[2 * D:][cs]),
            "wout": wout_perm.astype(bf),
            "bout": b_out,
            "ident": np.eye(128, dtype=np.float32).astype(bf),
        })
    return in_maps


def _install_ntff_hook():
    """The agent image's antenv lacks axon_hooks; shim it and register the
    ctypes NTFF profiler from trn_agent_boot so trace=True works."""
    import sys
    import types

    if "antenv.axon_hooks" in sys.modules:
        return
    import antenv

    mod = types.ModuleType("antenv.axon_hooks")
    mod._hook = None
    mod.set_axon_ntff_profile_hook = lambda h: setattr(mod, "_hook", h)
    mod.get_axon_ntff_profile_hook = lambda: mod._hook
    sys.modules["antenv.axon_hooks"] = mod
    antenv.axon_hooks = mod
    try:
        from trn_agent_boot.trn_boot import _ntff_profile_via_ctypes
        mod._hook = _ntff_profile_via_ctypes("/opt/axon/libaxon_pjrt.so")
    except Exception as e:
        print(f"ntff hook install failed: {e}")


def _run(inputs, trace=False):
    if trace:
        _install_ntff_hook()
    if "nc" not in _CACHE:
        _CACHE["nc"] = _build()
    nc = _CACHE["nc"]
    in_maps = _shard(inputs)
    r = bass_utils.run_bass_kernel_spmd(
        nc, in_maps, core_ids=list(range(NCORE)), trace=trace)
    out = np.empty((B, S, D), np.float32)
    for c in range(NCORE):
        for g in range(B):
            for qh in range(2):
                out[g, qh * 1024 + c * QB: qh * 1024 + (c + 1) * QB, :] = \
                    r.results[c]["out"][(g * 2 + qh) * QB:
                                        (g * 2 + qh + 1) * QB].astype(np.float32)
    return out, r


def kernel(**inputs) -> np.ndarray:
    out, _ = _run(inputs, trace=False)
    return out
